# revision 12
# baseline (speedup 1.0000x reference)
"""KDE-KNN kernel for Trainium2 (8 NeuronCores, SPMD).

Problem: for each of M=8192 points x_i (3-D), among points sharing its group id
(32 groups), find the K=16-th smallest euclidean distance w (self included),
then p_i = pi*w^2/(K-1), with fallback p_i = 1/c_i when the group count c_i < K.

Strategy:
  * Host: sort points by group id (pure layout work). Each group's members are
    then contiguous, so a point's candidate set is one contiguous column window.
  * Device: one bf16 matmul per 128-row tile computes the NEGATED squared
    distances  -d2[m,n] = 2*x_m.x_n - |x_m|^2 - |x_n|^2  with a K=15
    contraction: coords + norm terms folded into the operands, and fp32
    precision recovered via a hi/lo bf16 split
    ([A_hi;A_lo;A_hi]^T @ [B_hi;B_hi;B_lo], only the lo*lo term dropped).
    The 16-th smallest per row is then extracted straight out of PSUM with
    vector max8 -> match_replace8 -> max8 (3 passes). Since dim = ni-1 = 2,
    vol = pi*w^2 needs no sqrt:  p = pi/(K-1) * relu(d2_kth).
  * The 8 cores run one shared NEFF; all per-core differences live in the input
    data (each core gets its own pre-sliced tile operands).
  * Rows whose group has fewer than K members keep the reference fallback
    p = 1/c, applied on the host (their device value is well-defined garbage).
"""

import math

import ml_dtypes
import numpy as np

import concourse.bacc as bacc
import concourse.mybir as mybir
import concourse.tile as tile
from concourse.bass_utils import run_bass_kernel_spmd

M, NI, G = 8192, 3, 32
N_CORES = 8
P = 128  # partitions / rows per tile
BIG = 1.0e9  # negated-d2 offset for padding columns (below any real value)
NEG_INF = -3.0e38  # match_replace fill

LAST_RESULTS = None


def _plan_tiles(counts, starts):
    """One tile per (group, 128-row block): (row_start, nrows, win_start, win_len)."""
    tiles = []
    for g in range(len(counts)):
        c, s = int(counts[g]), int(starts[g])
        if c == 0:
            continue
        for r0 in range(0, c, P):
            tiles.append((s + r0, min(P, c - r0), s, c))
    return tiles


def _balance(tiles, n_cores):
    """Greedy least-loaded assignment by window length; returns per-core tile
    lists, each sorted by descending window so slot widths align across cores."""
    order = sorted(range(len(tiles)), key=lambda i: -tiles[i][3])
    loads = [0.0] * n_cores
    percore = [[] for _ in range(n_cores)]
    for i in order:
        c = loads.index(min(loads))
        percore[c].append(tiles[i])
        loads[c] += tiles[i][3] + 60.0  # fixed per-tile overhead term
    return percore


def _split_bf16(a):
    hi = a.astype(ml_dtypes.bfloat16)
    lo = (a - hi.astype(np.float32)).astype(ml_dtypes.bfloat16)
    return hi, lo


def kernel(x: np.ndarray, min_t_idx: np.ndarray, K) -> np.ndarray:
    x = np.asarray(x, dtype=np.float32)
    gid = np.asarray(min_t_idx)
    K = int(K)
    m = x.shape[0]
    assert x.shape == (m, NI) and gid.shape == (m,)

    # ---- host-side layout: sort by group --------------------------------
    perm = np.argsort(gid, kind="stable")
    gp = gid[perm]
    xp = x[perm]
    ngroups = int(gp[-1]) + 1 if m else 0
    counts = np.bincount(gp, minlength=ngroups)
    starts = np.concatenate([[0], np.cumsum(counts)[:-1]])

    sq = np.sum(xp * xp, axis=1, dtype=np.float32)
    # lhsT source rows: [x0, x1, x2, -sq, -1]; rhs source rows: [2x0, 2x1, 2x2, 1, sq]
    A = np.empty((5, m), dtype=np.float32)
    A[0:3] = xp.T
    A[3] = -sq
    A[4] = -1.0
    B = np.empty((5, m), dtype=np.float32)
    B[0:3] = 2.0 * xp.T
    B[3] = 1.0
    B[4] = sq
    A_hi, A_lo = _split_bf16(A)
    B_hi, B_lo = _split_bf16(B)
    # K=15 contraction recovers (hi+lo)*(hi+lo) up to the lo*lo term
    A15 = np.vstack([A_hi, A_lo, A_hi])  # [15, m] bf16
    B15 = np.vstack([B_hi, B_hi, B_lo])

    tiles = _plan_tiles(counts, starts)
    percore = _balance(tiles, N_CORES)
    T = max(len(tl) for tl in percore)
    # per-slot window width = max over cores (uniform program across cores)
    W = [
        max(tl[t][3] if t < len(tl) else 8 for tl in percore)
        for t in range(T)
    ]
    W = [min(512, max(8, (w + 3) & ~3)) for w in W]
    for tl in percore:
        for t, tt in enumerate(tl):
            assert tt[3] <= W[t], f"group window {tt[3]} exceeds slot width {W[t]}"
    offs = np.concatenate([[0], np.cumsum(W)]).astype(int)
    SW = int(offs[-1])

    # ---- per-core input marshaling --------------------------------------
    big_bf = ml_dtypes.bfloat16(BIG)
    TP = T * P
    in_maps = []
    for tl in percore:
        ab = np.zeros((15, TP + SW), dtype=ml_dtypes.bfloat16)
        lhs = ab[:, :TP]
        rhs = ab[:, TP:]
        rhs[4, :] = big_bf  # pad columns pair with lhsT row4 = -1 -> negd2 = -BIG
        for t, (row_start, nrows, win_start, win_len) in enumerate(tl):
            lhs[:, t * P : t * P + nrows] = A15[:, row_start : row_start + nrows]
            rhs[:, offs[t] : offs[t] + win_len] = B15[:, win_start : win_start + win_len]
        in_maps.append({"ab": ab})

    # ---- build the device program (shared by all cores) -----------------
    nc = bacc.Bacc("TRN2", target_bir_lowering=False, debug=False, num_devices=N_CORES)
    ab_d = nc.dram_tensor("ab", [15, TP + SW], mybir.dt.bfloat16, kind="ExternalInput")
    out_d = nc.dram_tensor("out", [T * P], mybir.dt.float32, kind="ExternalOutput")

    rounds = max(1, (K + 7) // 8)  # max8 rounds; match_replace between them
    last_col = (K - 1) - 8 * (rounds - 1)
    scale = -math.pi / max(K - 1, 1)

    with tile.TileContext(nc) as tc:
        with (
            tc.tile_pool(name="io", bufs=1) as io_pool,
            tc.tile_pool(name="small", bufs=4) as small_pool,
            tc.tile_pool(name="psum", bufs=6, space="PSUM") as psum_pool,
        ):
            ab_sb = io_pool.tile([15, TP + SW], mybir.dt.bfloat16)
            lhs_sb = ab_sb[:, :TP]
            rhs_sb = ab_sb[:, TP:]
            m8_all = io_pool.tile([P, T, 8], mybir.dt.float32)
            out_sb = io_pool.tile([P, T], mybir.dt.float32)
            nc.sync.dma_start(ab_sb[:], ab_d[:])

            for t in range(T):
                w = W[t]
                ps = psum_pool.tile([P, w], mybir.dt.float32, tag="ps")
                nc.tensor.matmul(
                    ps[:],
                    lhs_sb[:, t * P : (t + 1) * P],
                    rhs_sb[:, int(offs[t]) : int(offs[t]) + w],
                    start=True,
                    stop=True,
                )
                m8 = small_pool.tile([P, 8], mybir.dt.float32, tag="m8")
                for _ in range(rounds - 1):
                    nc.vector.max(out=m8[:], in_=ps[:])
                    nc.vector.match_replace(
                        out=ps[:], in_to_replace=m8[:], in_values=ps[:],
                        imm_value=NEG_INF,
                    )
                nc.vector.max(out=m8_all[:, t, :], in_=ps[:])

            # p = (pi/(K-1)) * relu(d2_kth); m8 holds -d2 so scale<0 then max 0
            nc.vector.tensor_scalar(
                out_sb[:],
                m8_all[:, :, last_col],
                float(scale),
                0.0,
                op0=mybir.AluOpType.mult,
                op1=mybir.AluOpType.max,
            )
            nc.sync.dma_start(out_d.rearrange("(t p) -> p t", p=P), out_sb[:])

    nc.compile()

    # If BASS_TRACE is set but this image's antenv lacks axon_hooks, inject a
    # None-returning stub so run_bass_kernel_spmd degrades to untraced.
    try:
        import antenv.axon_hooks  # noqa: F401
    except ImportError:
        import sys
        import types

        _m = types.ModuleType("antenv.axon_hooks")
        _m.get_axon_ntff_profile_hook = lambda: None
        _m.set_axon_ntff_profile_hook = lambda h: None
        sys.modules["antenv.axon_hooks"] = _m

    res = run_bass_kernel_spmd(nc, in_maps, core_ids=list(range(N_CORES)))
    global LAST_RESULTS
    LAST_RESULTS = res

    # ---- gather / unshard ----------------------------------------------
    p_perm = np.empty((m,), dtype=np.float32)
    for core, tl in enumerate(percore):
        o = res.results[core]["out"]
        for t, (row_start, nrows, _ws, _wl) in enumerate(tl):
            p_perm[row_start : row_start + nrows] = o[t * P : t * P + nrows]
    # reference fallback for rows whose group is smaller than K: p = 1/c
    crow = counts[gp]
    small = crow < K
    if small.any():
        p_perm[small] = (
            np.float32(1.0) / crow[small].astype(np.float32)
        ).astype(np.float32)
    p = np.empty((m,), dtype=np.float32)
    p[perm] = p_perm
    return p


# revision 15
# speedup vs baseline: 1.4088x; 1.4088x over previous
"""KDE-KNN kernel for Trainium2 (8 NeuronCores, SPMD).

Problem: for each of M=8192 points x_i (3-D), among points sharing its group id
(32 groups), find the K=16-th smallest euclidean distance w (self included),
then p_i = pi*w^2/(K-1), with fallback p_i = 1/c_i when the group count c_i < K.

Strategy:
  * Host: sort points by group id (pure layout work). Each group's members are
    then contiguous, so a point's candidate set is one contiguous column window.
  * Device: one bf16 matmul per 128-row tile computes the NEGATED squared
    distances  -d2[m,n] = 2*x_m.x_n - |x_m|^2 - |x_n|^2  with a K=15
    contraction: coords + norm terms folded into the operands, and fp32
    precision recovered via a hi/lo bf16 split
    ([A_hi;A_lo;A_hi]^T @ [B_hi;B_hi;B_lo], only the lo*lo term dropped).
    The 16-th smallest per row is then extracted straight out of PSUM with
    vector max8 -> match_replace8 -> max8 (3 passes). Since dim = ni-1 = 2,
    vol = pi*w^2 needs no sqrt:  p = pi/(K-1) * relu(d2_kth).
  * The 8 cores run one shared NEFF; all per-core differences live in the input
    data (each core gets its own pre-sliced tile operands).
  * Rows whose group has fewer than K members keep the reference fallback
    p = 1/c, applied on the host (their device value is well-defined garbage).
"""

import math

import ml_dtypes
import numpy as np

import concourse.bacc as bacc
import concourse.mybir as mybir
import concourse.tile as tile
from concourse.bass_utils import run_bass_kernel_spmd

M, NI, G = 8192, 3, 32
N_CORES = 8
P = 128  # partitions / rows per tile
BIG = 1.0e9  # negated-d2 offset for padding columns (below any real value)
NEG_INF = -3.0e38  # match_replace fill

LAST_RESULTS = None


def _plan_tiles(counts, starts):
    """One tile per (group, 128-row block): (row_start, nrows, win_start, win_len)."""
    tiles = []
    for g in range(len(counts)):
        c, s = int(counts[g]), int(starts[g])
        if c == 0:
            continue
        for r0 in range(0, c, P):
            tiles.append((s + r0, min(P, c - r0), s, c))
    return tiles


def _balance(tiles, n_cores):
    """Greedy least-loaded assignment by window length; returns per-core tile
    lists, each sorted by descending window so slot widths align across cores."""
    order = sorted(range(len(tiles)), key=lambda i: -tiles[i][3])
    loads = [0.0] * n_cores
    percore = [[] for _ in range(n_cores)]
    for i in order:
        c = loads.index(min(loads))
        percore[c].append(tiles[i])
        loads[c] += tiles[i][3] + 60.0  # fixed per-tile overhead term
    return percore


def _split_bf16(a):
    hi = a.astype(ml_dtypes.bfloat16)
    lo = (a - hi.astype(np.float32)).astype(ml_dtypes.bfloat16)
    return hi, lo


def kernel(x: np.ndarray, min_t_idx: np.ndarray, K) -> np.ndarray:
    x = np.asarray(x, dtype=np.float32)
    gid = np.asarray(min_t_idx)
    K = int(K)
    m = x.shape[0]
    assert x.shape == (m, NI) and gid.shape == (m,)

    # ---- host-side layout: sort by group --------------------------------
    perm = np.argsort(gid, kind="stable")
    gp = gid[perm]
    xp = x[perm]
    ngroups = int(gp[-1]) + 1 if m else 0
    counts = np.bincount(gp, minlength=ngroups)
    starts = np.concatenate([[0], np.cumsum(counts)[:-1]])

    sq = np.sum(xp * xp, axis=1, dtype=np.float32)
    # lhsT source rows: [x0, x1, x2, -sq, -1]; rhs source rows: [2x0, 2x1, 2x2, 1, sq]
    A = np.empty((5, m), dtype=np.float32)
    A[0:3] = xp.T
    A[3] = -sq
    A[4] = -1.0
    B = np.empty((5, m), dtype=np.float32)
    B[0:3] = 2.0 * xp.T
    B[3] = 1.0
    B[4] = sq
    A_hi, A_lo = _split_bf16(A)
    B_hi, B_lo = _split_bf16(B)
    # K=15 contraction recovers (hi+lo)*(hi+lo) up to the lo*lo term
    A15 = np.vstack([A_hi, A_lo, A_hi])  # [15, m] bf16
    B15 = np.vstack([B_hi, B_hi, B_lo])

    tiles = _plan_tiles(counts, starts)
    percore = _balance(tiles, N_CORES)
    T = max(len(tl) for tl in percore)
    # per-slot window width = max over cores (uniform program across cores)
    W = [
        max(tl[t][3] if t < len(tl) else 8 for tl in percore)
        for t in range(T)
    ]
    W = [min(512, max(8, (w + 3) & ~3)) for w in W]
    for tl in percore:
        for t, tt in enumerate(tl):
            assert tt[3] <= W[t], f"group window {tt[3]} exceeds slot width {W[t]}"
    offs = np.concatenate([[0], np.cumsum(W)]).astype(int)
    SW = int(offs[-1])

    # ---- per-core input marshaling --------------------------------------
    big_bf = ml_dtypes.bfloat16(BIG)
    TP = T * P
    in_maps = []
    for tl in percore:
        ab = np.zeros((15, TP + SW), dtype=ml_dtypes.bfloat16)
        lhs = ab[:, :TP]
        rhs = ab[:, TP:]
        rhs[4, :] = big_bf  # pad columns pair with lhsT row4 = -1 -> negd2 = -BIG
        for t, (row_start, nrows, win_start, win_len) in enumerate(tl):
            lhs[:, t * P : t * P + nrows] = A15[:, row_start : row_start + nrows]
            rhs[:, offs[t] : offs[t] + win_len] = B15[:, win_start : win_start + win_len]
        in_maps.append({"ab": ab})

    # ---- build the device program (shared by all cores) -----------------
    nc = bacc.Bacc("TRN2", target_bir_lowering=False, debug=False, num_devices=N_CORES)
    ab_d = nc.dram_tensor("ab", [15, TP + SW], mybir.dt.bfloat16, kind="ExternalInput")
    out_d = nc.dram_tensor("out", [P, T], mybir.dt.float32, kind="ExternalOutput")

    rounds = max(1, (K + 7) // 8)  # max8 rounds; match_replace between them
    last_col = (K - 1) - 8 * (rounds - 1)
    scale = -math.pi / max(K - 1, 1)

    with tile.TileContext(nc) as tc:
        with (
            tc.tile_pool(name="io", bufs=1) as io_pool,
            tc.tile_pool(name="small", bufs=4) as small_pool,
            tc.tile_pool(name="psum", bufs=6, space="PSUM") as psum_pool,
        ):
            ab_sb = io_pool.tile([15, TP + SW], mybir.dt.bfloat16)
            lhs_sb = ab_sb[:, :TP]
            rhs_sb = ab_sb[:, TP:]
            m8_all = io_pool.tile([P, T, 8], mybir.dt.float32)
            out_sb = io_pool.tile([P, T], mybir.dt.float32)
            nc.sync.dma_start(ab_sb[:], ab_d[:])

            for t in range(T):
                w = W[t]
                ps = psum_pool.tile([P, w], mybir.dt.float32, tag="ps")
                nc.tensor.matmul(
                    ps[:],
                    lhs_sb[:, t * P : (t + 1) * P],
                    rhs_sb[:, int(offs[t]) : int(offs[t]) + w],
                    start=True,
                    stop=True,
                )
                m8 = small_pool.tile([P, 8], mybir.dt.float32, tag="m8")
                for _ in range(rounds - 1):
                    nc.vector.max(out=m8[:], in_=ps[:])
                    nc.vector.match_replace(
                        out=ps[:], in_to_replace=m8[:], in_values=ps[:],
                        imm_value=NEG_INF,
                    )
                nc.vector.max(out=m8_all[:, t, :], in_=ps[:])

            # p = (pi/(K-1)) * relu(d2_kth); m8 holds -d2 so scale<0 then max 0
            nc.vector.tensor_scalar(
                out_sb[:],
                m8_all[:, :, last_col],
                float(scale),
                0.0,
                op0=mybir.AluOpType.mult,
                op1=mybir.AluOpType.max,
            )
            nc.sync.dma_start(out_d[:], out_sb[:])

    nc.compile()

    # If BASS_TRACE is set but this image's antenv lacks axon_hooks, inject a
    # None-returning stub so run_bass_kernel_spmd degrades to untraced.
    try:
        import antenv.axon_hooks  # noqa: F401
    except ImportError:
        import sys
        import types

        _m = types.ModuleType("antenv.axon_hooks")
        _m.get_axon_ntff_profile_hook = lambda: None
        _m.set_axon_ntff_profile_hook = lambda h: None
        sys.modules["antenv.axon_hooks"] = _m

    res = run_bass_kernel_spmd(nc, in_maps, core_ids=list(range(N_CORES)))
    global LAST_RESULTS
    LAST_RESULTS = res

    # ---- gather / unshard ----------------------------------------------
    p_perm = np.empty((m,), dtype=np.float32)
    for core, tl in enumerate(percore):
        o = res.results[core]["out"]  # [P, T]: column t holds slot t's rows
        for t, (row_start, nrows, _ws, _wl) in enumerate(tl):
            p_perm[row_start : row_start + nrows] = o[:nrows, t]
    # reference fallback for rows whose group is smaller than K: p = 1/c
    crow = counts[gp]
    small = crow < K
    if small.any():
        p_perm[small] = (
            np.float32(1.0) / crow[small].astype(np.float32)
        ).astype(np.float32)
    p = np.empty((m,), dtype=np.float32)
    p[perm] = p_perm
    return p
